# revision 16
# baseline (speedup 1.0000x reference)
"""Trainium2 Bass kernel for BaselineParameterizedPool2D.

Reference op: 3x3/stride-2/pad-1 max pool over xs [16,64,256,256] where each
of the 9 taps gets a per-(tap,channel) bias h[0,k,c] added before the max;
returns (pooled f32, argmax-tap-index int32), both [16,64,128,128].

Distribution: data-parallel over batch — 8 cores x 2 batches each.
Per-core layout: partitions = (b_local, c) = 2*64 = 128; free dim = spatial.

Per chunk of R=8 output rows:
  - DMA 17 input rows into SBUF [128, 17, 258] (col 0 = -10 pad).
  - ScalarE builds 9 biased tap planes T[128, 9(slot), R, 128] via
    activation(Identity, bias=h[k] per-partition, strided AP); slots hold
    taps in REVERSED order (slot s = tap 8-s).
  - VectorE: tensor_reduce(max) over tap axis -> pooled m.
            tensor_tensor(is_ge) T vs broadcast-m -> mask.
            copy_predicated(broadcast-out p, mask, iota-k) -> provenance;
            slots iterate 0..8 = taps 8..0, so the LAST predicated write is
            the SMALLEST winning tap = jnp.argmax first-match semantics.
  - DMA m, p out.
"""

import numpy as np

import concourse.bacc as bacc
import concourse.bass as bass
import concourse.mybir as mybir
from concourse.tile import TileContext

F32 = mybir.dt.float32
I32 = mybir.dt.int32

B = 16          # full batch
NCORES = 8
B_LOC = B // NCORES   # 2
C = 64
H = 256
W = 256
HO = 128
WO = 128
KS = 3
PAD = -10.0

R = 8                   # output rows per chunk
NCHUNK = HO // R        # 16
NR = 2 * R + 1          # input rows needed per chunk


def emit(nc: bass.Bass, nchunk: int = NCHUNK):
    xs_d = nc.dram_tensor("xs", [B_LOC, C, H, W], F32, kind="ExternalInput")
    h_d = nc.dram_tensor("h", [1, KS * KS, C], F32, kind="ExternalInput")
    pooled_d = nc.dram_tensor("pooled", [B_LOC, C, HO, WO], F32, kind="ExternalOutput")
    prov_d = nc.dram_tensor("prov", [B_LOC, C, HO, WO], I32, kind="ExternalOutput")

    xs_f = xs_d.ap().rearrange("b c h w -> (b c) h w")        # [128, 256, 256]
    pooled_f = pooled_d.ap().rearrange("b c h w -> (b c) h w")  # [128, 128, 128]
    prov_f = prov_d.ap().rearrange("b c h w -> (b c) h w")

    with TileContext(nc) as tc:
        with (
            tc.tile_pool(name="const", bufs=1) as constp,
            tc.tile_pool(name="io", bufs=2) as iop,
            tc.tile_pool(name="work", bufs=2) as workp,
        ):
            # h_sb[p, k] = h[0, k, p % 64] : per-partition bias columns
            h_sb = constp.tile([128, KS * KS], F32)
            h_src = h_d.ap()[0].transpose([1, 0])   # [64, 9]
            nc.sync.dma_start(h_sb[0:64, :], h_src)
            nc.sync.dma_start(h_sb[64:128, :], h_src)

            # ktile[p, s] = 8 - s  (tap index stored at slot s), as f32 —
            # copy_predicated is float-only per the BIR verifier
            ktile_i = constp.tile([128, KS * KS], I32)
            nc.gpsimd.iota(ktile_i[:], pattern=[[-1, KS * KS]], base=8,
                           channel_multiplier=0)
            ktile = constp.tile([128, KS * KS], F32)
            nc.vector.tensor_copy(ktile[:], ktile_i[:])

            # persistent ping-pong input tiles: pad columns/rows memset once
            xin0 = constp.tile([128, NR + 1, 258], F32)
            xin1 = constp.tile([128, NR + 1, 258], F32)
            xin_bufs = [xin0, xin1]
            nc.gpsimd.memset(xin0[:, :, 0:1], PAD)
            nc.gpsimd.memset(xin1[:, :, 0:1], PAD)
            nc.gpsimd.memset(xin0[:, 0:1, :], PAD)   # row -1 pad (chunk 0)

            # collapse all setup waits so per-chunk ops carry few sync slots
            tc.strict_bb_all_engine_barrier()

            for ch in range(nchunk):
                xin = xin_bufs[ch % 2]
                r0 = 2 * ch * R - 1   # first input row of this chunk
                if ch == 0:
                    nc.gpsimd.dma_start(xin[:, 1:NR, 1:257], xs_f[:, 0:NR - 1, :])
                else:
                    nc.gpsimd.dma_start(xin[:, 0:NR, 1:257], xs_f[:, r0:r0 + NR, :])

                # 9 biased taps, slot s = tap 8-s, on ScalarE
                T = workp.tile([128, KS * KS, R, WO], F32, tag="T")
                for s in range(KS * KS):
                    k = 8 - s
                    di, dj = divmod(k, 3)
                    src = xin[:, di:di + 2 * R:2, dj:dj + 2 * WO:2]
                    nc.scalar.activation(
                        T[:, s], src,
                        mybir.ActivationFunctionType.Identity,
                        bias=h_sb[:, k:k + 1], scale=1.0,
                    )

                # pooled = max over taps
                m = iop.tile([128, R, WO], F32, tag="m")
                nc.vector.tensor_reduce(
                    m[:], T[:].transpose([0, 2, 3, 1]),
                    axis=mybir.AxisListType.X, op=mybir.AluOpType.max,
                )

                # mask = (T >= m) per tap
                msk = workp.tile([128, KS * KS, R, WO], mybir.dt.uint32,
                                 tag="msk", bufs=1)
                m_b = m[:].unsqueeze(1).broadcast_to([128, KS * KS, R, WO])
                nc.vector.tensor_tensor(msk[:], T[:], m_b,
                                        op=mybir.AluOpType.is_ge)

                # provenance: per-slot predicated writes; slots ascend 0..8 =
                # taps 8..0, so the last write is the smallest winning tap
                # (matches jnp.argmax first-occurrence semantics).
                pf = iop.tile([128, R, WO], F32, tag="pf")
                nc.gpsimd.memset(pf[:], 0.0)
                for s in range(KS * KS):
                    k_b = ktile[:, s:s + 1].broadcast_to([128, R * WO])
                    nc.vector.copy_predicated(
                        pf[:].rearrange("p r w -> p (r w)"),
                        msk[:, s].rearrange("p r w -> p (r w)"), k_b)
                p = iop.tile([128, R, WO], I32, tag="p")
                nc.vector.tensor_copy(p[:], pf[:])

                nc.sync.dma_start(pooled_f[:, ch * R:(ch + 1) * R, :], m[:])
                nc.sync.dma_start(prov_f[:, ch * R:(ch + 1) * R, :], p[:])
    return nc


def build_nc(nchunk: int = NCHUNK, compile: bool = True):
    nc = bacc.Bacc("TRN2", target_bir_lowering=False, debug=False)
    emit(nc, nchunk=nchunk)
    if compile:
        nc.compile()
    return nc


def kernel(xs: np.ndarray, h: np.ndarray):
    from concourse.bass_utils import run_bass_kernel_spmd

    xs = np.ascontiguousarray(xs, dtype=np.float32)
    h = np.ascontiguousarray(h, dtype=np.float32)
    nc = build_nc()
    in_maps = [
        {"xs": np.ascontiguousarray(xs[i * B_LOC:(i + 1) * B_LOC]), "h": h}
        for i in range(NCORES)
    ]
    res = run_bass_kernel_spmd(nc, in_maps, core_ids=list(range(NCORES)))
    pooled = np.concatenate([r["pooled"] for r in res.results], axis=0)
    prov = np.concatenate([r["prov"] for r in res.results], axis=0)
    return pooled, prov
